# revision 2
# baseline (speedup 1.0000x reference)
"""MoE top-2 routing + expert FFN for Trainium2, expert-parallel across 8 cores.

v6 (over v4): the repeat loop is a 2-stage software pipeline
(For_i_pipelined([load, compute], unroll=2)) — iteration i+1's xt DMA runs
during iteration i's compute into the other half of a double-buffered xt, so
the PE never waits for the activation load at the loop back edge. y stores on
the GpSimd SWDGE queue, weight prefetch on Sync.

Contract: kernel(**inputs) takes the FULL unsharded inputs (numpy) and
returns the FULL output [B, S, D] float32.
"""

import os
import numpy as np
from contextlib import ExitStack

import ml_dtypes

B, S, D = 4, 2048, 1024
E, H, TOP_K = 8, 4096, 2
T = B * S
P = 128
KS1 = D // P          # 8  k-subtiles for the first matmul
HH = 2                # H halves
HML = H // HH // P    # 16 h-chunks per half
DC = D // P           # 8  output d-chunks
BF16 = ml_dtypes.bfloat16


def _routing(xf, Wr, br):
    """Bit-identical replication of the reference's routing on jax CPU."""
    import jax
    import jax.numpy as jnp

    cpu = jax.local_devices(backend="cpu")[0]
    with jax.default_device(cpu):
        gate = jax.nn.softmax(jnp.asarray(xf) @ jnp.asarray(Wr) + jnp.asarray(br), axis=-1)
        top_w, top_i = jax.lax.top_k(gate, TOP_K)
        top_w = top_w / jnp.sum(top_w, axis=-1, keepdims=True)
    return np.asarray(top_i), np.asarray(top_w)


def _subs(C):
    """512-col pieces grouped into sub-blocks of <=3 pieces (<=3 PSUM banks)."""
    pieces, off = [], 0
    while off < C:
        take = min(512, C - off)
        pieces.append((off, take))
        off += take
    return [pieces[i : i + 3] for i in range(0, len(pieces), 3)]


def _build_program(C, repeats):
    import concourse.tile as tile
    from concourse import bacc, mybir

    F32 = mybir.dt.float32
    BF = mybir.dt.bfloat16
    subs = _subs(C)

    nc = bacc.Bacc("TRN2", target_bir_lowering=False, debug=False, num_devices=E)

    xt_ap = nc.dram_tensor("xt", [P, KS1, C], BF, kind="ExternalInput").ap()
    w1_ap = nc.dram_tensor("w1", [HH, HML, P, KS1, P], BF, kind="ExternalInput").ap()
    w2_ap = nc.dram_tensor("w2", [HH, DC, P, HML, P], BF, kind="ExternalInput").ap()
    b1_ap = nc.dram_tensor("b1", [HH, HML, P], F32, kind="ExternalInput").ap()
    b2_ap = nc.dram_tensor("b2", [DC, P], F32, kind="ExternalInput").ap()
    y_ap = nc.dram_tensor("y", [P, DC, C], BF, kind="ExternalOutput").ap()

    with tile.TileContext(nc) as tc, ExitStack() as ctx:
        xt_pool = ctx.enter_context(tc.tile_pool(name="xt", bufs=1))
        h1_pool = ctx.enter_context(tc.tile_pool(name="h1", bufs=1))
        y_pool = ctx.enter_context(tc.tile_pool(name="y", bufs=1))
        w1_pool = ctx.enter_context(tc.tile_pool(name="w1", bufs=4))
        w2_pool = ctx.enter_context(tc.tile_pool(name="w2", bufs=3))
        bias_pool = ctx.enter_context(tc.tile_pool(name="bias", bufs=1))
        psA = ctx.enter_context(tc.tile_pool(name="psA", bufs=4, space="PSUM"))
        psB = ctx.enter_context(tc.tile_pool(name="psB", bufs=4, space="PSUM"))

        b1t = bias_pool.tile([P, HH * HML], F32)
        nc.sync.dma_start(b1t[:], b1_ap.rearrange("hh m p -> p (hh m)"))
        b2t = bias_pool.tile([P, DC], F32)
        nc.sync.dma_start(b2t[:], b2_ap.rearrange("d p -> p d"))

        def compute_body(xt):
            yt = y_pool.tile([P, DC, C], BF, tag="y", name="yt")
            for hh in range(HH):
                h1 = h1_pool.tile([P, HML, C], BF, tag="h1", name="h1")
                for m in range(HML):
                    w1t = w1_pool.tile([P, KS1, P], BF, tag="w1", name="w1t")
                    nc.sync.dma_start(w1t[:], w1_ap[hh, m])
                    for sub in subs:
                        banks = [
                            psA.tile([P, 512], F32, tag="psA", name="psA")[:, :pn]
                            for _, pn in sub
                        ]
                        for k in range(KS1):
                            for bi, (po, pn) in enumerate(sub):
                                nc.tensor.matmul(
                                    banks[bi],
                                    w1t[:, k, :],
                                    xt[:, k, po : po + pn],
                                    start=(k == 0),
                                    stop=(k == KS1 - 1),
                                )
                        for bi, (po, pn) in enumerate(sub):
                            nc.scalar.activation(
                                h1[:, m, po : po + pn],
                                banks[bi],
                                mybir.ActivationFunctionType.Gelu,
                                bias=b1t[:, hh * HML + m : hh * HML + m + 1],
                            )
                for d in range(DC):
                    w2t = w2_pool.tile([P, HML, P], BF, tag="w2", name="w2t")
                    nc.sync.dma_start(w2t[:], w2_ap[hh, d])
                    for sub in subs:
                        banks = [
                            psB.tile([P, 512], F32, tag="psB", name="psB")[:, :pn]
                            for _, pn in sub
                        ]
                        for k in range(HML):
                            for bi, (po, pn) in enumerate(sub):
                                nc.tensor.matmul(
                                    banks[bi],
                                    w2t[:, k, :],
                                    h1[:, k, po : po + pn],
                                    start=(k == 0),
                                    stop=(k == HML - 1),
                                )
                        for bi, (po, pn) in enumerate(sub):
                            if hh == 0:
                                nc.scalar.activation(
                                    yt[:, d, po : po + pn],
                                    banks[bi],
                                    mybir.ActivationFunctionType.Identity,
                                    bias=b2t[:, d : d + 1],
                                )
                            else:
                                nc.vector.tensor_tensor(
                                    yt[:, d, po : po + pn],
                                    banks[bi],
                                    yt[:, d, po : po + pn],
                                    mybir.AluOpType.add,
                                )
                    if hh == 1:
                        # SWDGE queue: keeps the Sync HWDGE queue free for the
                        # pipelined xt/weight prefetch.
                        nc.gpsimd.dma_start(y_ap[:, d, :], yt[:, d, :])

        if repeats > 1:
            from concourse import mybir as _mybir

            def load(pipe, iv):
                xt = pipe.intermediate_tile([P, KS1, C], BF, name="xt")
                nc.sync.dma_start(xt[:], xt_ap)
                return xt

            def compute(pipe, iv, xt):
                compute_body(xt)

            tc.For_i_pipelined(
                [load, compute],
                0,
                repeats,
                unroll=2,
                hint_engines=(
                    _mybir.EngineType.PE,
                    _mybir.EngineType.Activation,
                    _mybir.EngineType.SP,
                    _mybir.EngineType.DVE,
                    _mybir.EngineType.Pool,
                ),
            )
        else:
            xt = xt_pool.tile([P, KS1, C], BF, tag="xt", name="xt")
            nc.sync.dma_start(xt[:], xt_ap)
            compute_body(xt)

    nc.finalize()
    return nc


def _pack_inputs(xf, W1, b1, W2, b2, top_i, top_w, C):
    """Per-expert gather + weight prepack into the device tile layouts (bf16)."""
    in_maps = []
    idx_list = []
    w_list = []
    W1b = W1.astype(BF16)
    W2b = W2.astype(BF16)
    for e in range(E):
        sel = (top_i == e).any(axis=1)
        idx = np.nonzero(sel)[0]
        we = (top_w * (top_i == e)).sum(axis=1)[idx].astype(np.float32)
        idx_list.append(idx)
        w_list.append(we)

        n = len(idx)
        Xg = np.zeros((C, D), dtype=np.float32)
        Xg[:n] = xf[idx]
        # [P, KS1, C]: partition p = d % 128, k-subtile ks = d // 128
        xt = np.ascontiguousarray(Xg.reshape(C, KS1, P).transpose(2, 1, 0)).astype(BF16)

        w1p = np.ascontiguousarray(
            W1b[e].reshape(KS1, P, HH, HML, P).transpose(2, 3, 1, 0, 4)
        )
        w2p = np.ascontiguousarray(
            W2b[e].reshape(HH, HML, P, DC, P).transpose(0, 3, 2, 1, 4)
        )
        b1p = np.ascontiguousarray(b1[e].reshape(HH, HML, P)).astype(np.float32)
        b2p = np.ascontiguousarray(b2[e].reshape(DC, P)).astype(np.float32)

        in_maps.append({"xt": xt, "w1": w1p, "w2": w2p, "b1": b1p, "b2": b2p})
    return in_maps, idx_list, w_list


def _run(x, Wr, br, W1, b1, W2, b2, repeats=1, timing_runs=0):
    import time

    from concourse.bass_utils import run_bass_kernel_spmd

    x = np.asarray(x, dtype=np.float32)
    Wr = np.asarray(Wr, dtype=np.float32)
    br = np.asarray(br, dtype=np.float32)
    W1 = np.asarray(W1, dtype=np.float32)
    b1 = np.asarray(b1, dtype=np.float32)
    W2 = np.asarray(W2, dtype=np.float32)
    b2 = np.asarray(b2, dtype=np.float32)

    xf = x.reshape(T, D)
    top_i, top_w = _routing(xf, Wr, br)

    counts = np.bincount(top_i.ravel(), minlength=E)
    max_count = int(counts.max())
    C = max(256, int(np.ceil(max_count / 8)) * 8)

    nc = _build_program(C, repeats)
    in_maps, idx_list, w_list = _pack_inputs(xf, W1, b1, W2, b2, top_i, top_w, C)

    res = run_bass_kernel_spmd(nc, in_maps, core_ids=list(range(E)))

    walls = []
    for _ in range(timing_runs):
        t0 = time.perf_counter()
        run_bass_kernel_spmd(nc, in_maps, core_ids=list(range(E)))
        walls.append(time.perf_counter() - t0)

    out = np.zeros((T, D), dtype=np.float32)
    for e in range(E):
        idx = idx_list[e]
        n = len(idx)
        if n == 0:
            continue
        # y is [P, DC, C] with d = dc * 128 + p
        yp = res.results[e]["y"].astype(np.float32)
        Ye = yp.transpose(2, 1, 0).reshape(-1, D)  # [C, D]
        out[idx] += w_list[e][:, None] * Ye[:n]

    return out.reshape(B, S, D), walls


def kernel(x, Wr, br, W1, b1, W2, b2):
    out, _ = _run(x, Wr, br, W1, b1, W2, b2, repeats=1)
    return out
